# revision 2
# baseline (speedup 1.0000x reference)
import os
import sys

import numpy as np

for _p in ("/opt/trn_rl_repo", "/root/.axon_site/_ro/trn_rl_repo"):
    if os.path.isdir(_p) and _p not in sys.path:
        sys.path.insert(0, _p)

import concourse.tile as tile
from concourse import bacc, mybir

# Problem: y = causal dilated conv1d (C=64->64, K=2, dilation=64) over x[16,64,16384],
# then tanh(y)*sigmoid(y).  Sharded data-parallel over batch: 2 batches per core.
#
# v2: 16-bit I/O end to end.  The harness gate is rel_err < 2e-2; fp16 x/w/y with
# fp32 PSUM accumulation lands ~1e-3 and halves HBM traffic vs the fp32 baseline.
B, C, T = 16, 64, 16384
KERNEL = 2
DIL = 64
N_CORES = 8
B_PER = B // N_CORES  # 2
P = B_PER * C  # 128 partitions: batch 0 on 0..63, batch 1 on 64..127
NT = int(os.environ.get("KERNEL_NT", "2048"))  # time-tile (columns per DMA)
CHUNK = 512  # PSUM bank free size (fp32)
MM_FD = int(os.environ.get("KERNEL_MM_FD", "512"))  # cols per matmul (<=1024 16-bit)
ACT_FD = int(os.environ.get("KERNEL_ACT_FD", "2048"))  # cols per ACTIVATE block
F32 = mybir.dt.float32
IO_DTYPE = os.environ.get("KERNEL_IO_DTYPE", "float16")


def _build_program():
    nc = bacc.Bacc("TRN2", target_bir_lowering=False, debug=False)
    iodt = getattr(mybir.dt, IO_DTYPE)
    x_in = nc.dram_tensor("x", [B_PER, C, T], iodt, kind="ExternalInput")
    # Host-preprocessed weights: wt[k] is the 128x128 block-diagonal stationary
    # matrix for tap k (two copies of w[:,:,k].T on the diagonal), so one K=128
    # matmul computes both batches' 64x64 channel mix.
    wt_in = nc.dram_tensor("wt", [KERNEL, P, P], iodt, kind="ExternalInput")
    y_out = nc.dram_tensor("y", [B_PER, C, T], iodt, kind="ExternalOutput")

    x_flat = x_in[:].flatten_outer_dims()  # [128, T]
    y_flat = y_out[:].flatten_outer_dims()  # [128, T]

    with tile.TileContext(nc) as tc:
        with (
            tc.tile_pool(name="wpool", bufs=1) as wpool,
            tc.tile_pool(name="xpool", bufs=4) as xpool,
            tc.tile_pool(name="opool", bufs=4) as opool,
            tc.tile_pool(name="actpool", bufs=4) as actpool,
            tc.tile_pool(name="psum", bufs=max(2, 4096 // ACT_FD), space="PSUM") as psumpool,
        ):
            # tiny weight loads first so the first matmuls are gated only on
            # the first x tile, not on late weight DMAs
            wblk = []
            for k in range(KERNEL):
                wk = wpool.tile([P, P], iodt, tag=f"w{k}")
                nc.sync.dma_start(out=wk[:], in_=wt_in[k])
                wblk.append(wk)

            # first/last tiles are half-size: the first matmuls wait on a
            # smaller first DMA, and the final act->mul->store drain is shorter
            EDGE = NT // 2
            tiles = (
                [(0, EDGE)]
                + [(EDGE + i * NT, NT) for i in range((T - 2 * EDGE) // NT)]
                + [(T - EDGE, EDGE)]
            )

            xt0 = xpool.tile([P, EDGE + DIL], iodt, tag="xt")
            nc.vector.memset(xt0[:, 0:DIL], 0.0)
            nc.sync.dma_start(out=xt0[:, DIL:], in_=x_flat[:, 0:EDGE])

            # prime the ACT function tables on a dummy element so the ~2.6us
            # of ACT_TABLE_LOADs overlap the first input DMA
            prime = wpool.tile([1, 2], F32, tag="prime")
            nc.vector.memset(prime[:], 0.0)
            nc.scalar.activation(
                out=prime[:, 0:1],
                in_=prime[:, 1:2],
                func=mybir.ActivationFunctionType.Tanh,
            )
            nc.scalar.activation(
                out=prime[:, 0:1],
                in_=prime[:, 1:2],
                func=mybir.ActivationFunctionType.Sigmoid,
            )

            n_tiles = len(tiles)
            for it, (t0, nt) in enumerate(tiles):
                if it == 0:
                    xt = xt0
                else:
                    # x tile carries a DIL-column left halo: col j = t0 - DIL + j
                    xt = xpool.tile([P, nt + DIL], iodt, tag="xt")
                    nc.sync.dma_start(out=xt[:], in_=x_flat[:, t0 - DIL : t0 + nt])

                fd_act = min(ACT_FD, nt)
                base = 0
                for fd in [fd_act] * (nt // fd_act):
                    # y[t] = W1^T @ x[t]  +  W0^T @ x[t-DIL]
                    # tap-outer, chunk-inner: the stationary weights switch
                    # twice per block instead of per chunk
                    ps = psumpool.tile([P, fd], F32, tag="ps")
                    for k in (1, 0):
                        for c in range(0, fd, MM_FD):
                            nc.tensor.matmul(
                                out=ps[:, c : c + MM_FD],
                                lhsT=wblk[k][:],
                                rhs=xt[
                                    :,
                                    base + c + k * DIL : base + c + k * DIL + MM_FD,
                                ],
                                start=(k == 1),
                                stop=(k == 0),
                            )
                    th = actpool.tile([P, fd], iodt, tag="th")
                    sg = actpool.tile([P, fd], iodt, tag="sg")
                    nc.scalar.activation(
                        out=th[:], in_=ps[:], func=mybir.ActivationFunctionType.Tanh
                    )
                    nc.scalar.activation(
                        out=sg[:], in_=ps[:], func=mybir.ActivationFunctionType.Sigmoid
                    )
                    ot = opool.tile([P, fd], iodt, tag="ot")
                    nc.vector.tensor_mul(ot[:], th[:], sg[:])
                    # per-block output DMA from gpsimd: stores start as soon
                    # as each block's multiply lands (gpsimd keeps them off
                    # the sync ring, whose FIFO carries the input stream)
                    nc.gpsimd.dma_start(
                        out=y_flat[:, t0 + base : t0 + base + fd], in_=ot[:]
                    )
                    base += fd
    nc.finalize()
    return nc


def _host_weights(w: np.ndarray, np_dtype) -> np.ndarray:
    wt = np.zeros((KERNEL, P, P), dtype=np_dtype)
    for k in range(KERNEL):
        wTk = np.ascontiguousarray(w[:, :, k].T).astype(np_dtype)  # [ci, co]
        for b in range(B_PER):
            wt[k, b * C : (b + 1) * C, b * C : (b + 1) * C] = wTk
    return wt


def _ensure_ntff_hook():
    """Recreate the antenv.axon_hooks NTFF profiling shim if the image lacks it."""
    import types

    try:
        import antenv.axon_hooks  # noqa: F401

        return
    except ImportError:
        pass
    import antenv

    mod = types.ModuleType("antenv.axon_hooks")
    _h = {"hook": None}
    mod.set_axon_ntff_profile_hook = lambda h: _h.__setitem__("hook", h)
    mod.get_axon_ntff_profile_hook = lambda: _h["hook"]
    sys.modules["antenv.axon_hooks"] = mod
    antenv.axon_hooks = mod
    try:
        from trn_agent_boot.trn_boot import _ntff_profile_via_ctypes

        hook = _ntff_profile_via_ctypes("/opt/axon/libaxon_pjrt.so")
        if hook is not None:
            mod.set_axon_ntff_profile_hook(hook)
    except Exception as e:  # degrade to no-trace rather than crash
        print(f"ntff hook setup failed: {e}", file=sys.stderr)


def _run_spmd(x: np.ndarray, w: np.ndarray, trace: bool = False):
    from concourse import bass_utils
    from concourse.bass_utils import run_bass_kernel_spmd

    if trace:
        _ensure_ntff_hook()
        bass_utils.upload_artifacts = lambda tmpdir: tmpdir

    nc = _build_program()
    np_dtype = {"float16": np.float16, "bfloat16": None}.get(IO_DTYPE, np.float16)
    if IO_DTYPE == "bfloat16":
        import ml_dtypes

        np_dtype = ml_dtypes.bfloat16
    xio = np.ascontiguousarray(x.astype(np_dtype))
    wt = _host_weights(w, np_dtype)
    in_maps = [
        {"x": np.ascontiguousarray(xio[i * B_PER : (i + 1) * B_PER]), "wt": wt}
        for i in range(N_CORES)
    ]
    kwargs = {}
    if trace:
        import tempfile

        os.makedirs("/tmp/kernel_trace", exist_ok=True)
        kwargs["tmpdir"] = tempfile.mkdtemp(dir="/tmp/kernel_trace")
    res = run_bass_kernel_spmd(nc, in_maps, list(range(N_CORES)), trace=trace, **kwargs)
    y = np.concatenate(
        [res.results[i]["y"].astype(np.float32) for i in range(N_CORES)], axis=0
    )
    return y, res


def kernel(x: np.ndarray, w: np.ndarray) -> np.ndarray:
    x = np.ascontiguousarray(np.asarray(x, dtype=np.float32))
    w = np.ascontiguousarray(np.asarray(w, dtype=np.float32))
    trace = os.environ.get("KERNEL_TRACE", "0") == "1"
    y, res = _run_spmd(x, w, trace=trace)
    if trace:
        global LAST_RESULTS
        LAST_RESULTS = res
    return y


LAST_RESULTS = None


# revision 6
# speedup vs baseline: 1.1418x; 1.1418x over previous
import os
import sys

import numpy as np

for _p in ("/opt/trn_rl_repo", "/root/.axon_site/_ro/trn_rl_repo"):
    if os.path.isdir(_p) and _p not in sys.path:
        sys.path.insert(0, _p)

import concourse.tile as tile
from concourse import bacc, mybir

# Problem: y = causal dilated conv1d (C=64->64, K=2, dilation=64) over x[16,64,16384],
# then f(y) = tanh(y)*sigmoid(y).  Sharded data-parallel over batch: 2 batches/core.
#
# All HBM I/O is fp16 (harness gate is rel_err < 2e-2; this lands ~1e-3) which
# halves DMA vs fp32.  The gate is then ACT-bound (2 LUT passes/elem, ~1.15ns/elem,
# no 16-bit accel), so a fraction of blocks take "route B": one ACT pass
# t = tanh(y/2), then DVE finishes  f = (t+t^2) * P(t^2),  P(s) ~= 1/(1+s)
# (tanh(y) = 2t/(1+t^2), sigmoid(y) = (1+t)/2).  Route A/B split balances the
# Scalar and Vector engines.  Dummy matmuls at t=0 warm the PE HAM clock gate
# (cold PE runs matmuls at 1.2GHz vs 2.4GHz warm).
B, C, T = 16, 64, 16384
KERNEL = 2
DIL = 64
N_CORES = 8
B_PER = B // N_CORES  # 2
P = B_PER * C  # 128 partitions: batch 0 on 0..63, batch 1 on 64..127
BLK = 1536  # psum block (3 PSUM banks); 2 in flight + 1 dummy bank = 7 of 8
EDGE = 512  # first/last block size (short pipeline head/tail)
MM_FD = 512  # cols per matmul (PSUM bank limit for fp32 out)
N_MID = (T - 2 * EDGE) // BLK  # 10 middle blocks -> 5 pairs
F32 = mybir.dt.float32
IO_DTYPE = os.environ.get("KERNEL_IO_DTYPE", "float16")
# route-B pair indices (of 5 middle pairs); balance ACT vs DVE busy
B_PAIRS = {
    int(v) for v in os.environ.get("KERNEL_BPAIRS", "1,3").split(",") if v != ""
}
BDEG = int(os.environ.get("KERNEL_BDEG", "3"))
# P(s) ~= 1/(1+s) on s in [0,1]; highest-degree first
BCOEF = {
    3: (-0.23548745, 0.68627748, -0.95078937, 0.99873732),  # minimax, err 1.3e-3
    2: (0.28671682, -0.7644057, 0.9776889),  # norm-weighted, err 7e-3
}[BDEG]
N_PREWARM = int(os.environ.get("KERNEL_PREWARM", "18"))
Tanh = mybir.ActivationFunctionType.Tanh
Sigmoid = mybir.ActivationFunctionType.Sigmoid
Mult = mybir.AluOpType.mult
Add = mybir.AluOpType.add


def _build_program():
    nc = bacc.Bacc("TRN2", target_bir_lowering=False, debug=False)
    iodt = getattr(mybir.dt, IO_DTYPE)
    x_in = nc.dram_tensor("x", [B_PER, C, T], iodt, kind="ExternalInput")
    # Host-preprocessed weights: wt[k] is the 128x128 block-diagonal stationary
    # matrix for tap k (two copies of w[:,:,k].T on the diagonal), so one K=128
    # matmul computes both batches' 64x64 channel mix.
    wt_in = nc.dram_tensor("wt", [KERNEL, P, P], iodt, kind="ExternalInput")
    y_out = nc.dram_tensor("y", [B_PER, C, T], iodt, kind="ExternalOutput")

    x_flat = x_in[:].flatten_outer_dims()  # [128, T]
    y_flat = y_out[:].flatten_outer_dims()  # [128, T]

    with tile.TileContext(nc) as tc:
        with (
            tc.tile_pool(name="wpool", bufs=1) as wpool,
            tc.tile_pool(name="xpool", bufs=3) as xpool,
            tc.tile_pool(name="apool", bufs=2) as apool,
            tc.tile_pool(name="bpool", bufs=2) as bpool,
            tc.tile_pool(name="opool", bufs=3) as opool,
            tc.tile_pool(name="psum", bufs=2, space="PSUM") as psumpool,
            tc.tile_pool(name="psdum", bufs=1, space="PSUM") as psdumpool,
        ):
            # PE prewarm: the HAM clock gate needs ~3.4us of sustained PE
            # activity to lift the PE from 1.2GHz to 2.4GHz.  Burn dummy
            # matmuls on a spare PSUM bank while the first input DMA flies.
            wdum = wpool.tile([P, 128], iodt, tag="wdum")
            nc.vector.memset(wdum[:], 0.0078125)
            psd = psdumpool.tile([P, MM_FD], F32, tag="psd")
            for i in range(N_PREWARM):
                nc.tensor.matmul(
                    out=psd[:, 0:128], lhsT=wdum[:], rhs=wdum[:],
                    start=True, stop=True,
                )

            # tiny weight loads early so the first real matmuls are gated only
            # on the first x tile
            wblk = []
            for k in range(KERNEL):
                wk = wpool.tile([P, P], iodt, tag=f"w{k}")
                nc.sync.dma_start(out=wk[:], in_=wt_in[k])
                wblk.append(wk)

            # prime the ACT tables (tanh+sigmoid share a set once
            # sigmoid_and_others is resident) so the ~2.6us of table loads
            # overlap the first input DMA
            prime = wpool.tile([1, 2], F32, tag="prime")
            nc.vector.memset(prime[:], 0.0)
            nc.scalar.activation(out=prime[:, 0:1], in_=prime[:, 1:2], func=Tanh)
            nc.scalar.activation(out=prime[:, 0:1], in_=prime[:, 1:2], func=Sigmoid)

            # DMA tiles: [edge] + 5x[pair of BLK] + [edge]; halo of DIL cols
            xt0 = xpool.tile([P, EDGE + DIL], iodt, tag="xt_e")
            nc.vector.memset(xt0[:, 0:DIL], 0.0)
            nc.sync.dma_start(out=xt0[:, DIL:], in_=x_flat[:, 0:EDGE])

            def run_blocks(xt, t0, widths, dest, dest_off):
                """Matmuls + ACT for consecutive blocks in one x tile.

                xt col j = x[t0 - DIL + j].  For each block (offset into this
                tile), emit conv matmuls into one psum tile, then the ACT
                pass(es) writing into dest[:, dest_off:...].  dest rows are
                (func, out_tile) pairs: route A = [(Tanh, th), (Sigmoid, sg)],
                route B = [(tanh(y/2), t)].
                """
                off = dest_off
                base = 0
                for w, passes in widths:
                    ps = psumpool.tile([P, BLK], F32, tag="ps")
                    for k in (1, 0):
                        for c in range(0, w, MM_FD):
                            nc.tensor.matmul(
                                out=ps[:, c : c + MM_FD],
                                lhsT=wblk[k][:],
                                rhs=xt[:, base + c + k * DIL : base + c + k * DIL + MM_FD],
                                start=(k == 1),
                                stop=(k == 0),
                            )
                    # heartbeat: keep the PE HAM-warm through the ACT-bound
                    # stretch (psdum is always free, so these run in PE gaps)
                    for _ in range(2):
                        nc.tensor.matmul(
                            out=psd[:, 0:128], lhsT=wdum[:], rhs=wdum[:],
                            start=True, stop=True,
                        )
                    for func, scale, dst in passes:
                        nc.scalar.activation(
                            out=dst[:, off : off + w], in_=ps[:, 0:w],
                            func=func, scale=scale,
                        )
                    off += w
                    base += w

            def route_a_finish(th, sg, width, t0):
                ot = opool.tile([P, width], iodt, tag="ot")
                nc.vector.tensor_mul(ot[:], th[:, 0:width], sg[:, 0:width])
                nc.gpsimd.dma_start(out=y_flat[:, t0 : t0 + width], in_=ot[:])

            def route_b_finish(t, width, t0):
                c = BCOEF
                s = bpool.tile([P, width], iodt, tag="bs")
                nc.vector.tensor_mul(s[:], t[:, 0:width], t[:, 0:width])
                m = bpool.tile([P, width], iodt, tag="bm")
                nc.vector.tensor_add(m[:], t[:, 0:width], s[:])
                u = bpool.tile([P, width], iodt, tag="bu")
                nc.vector.tensor_scalar(
                    out=u[:], in0=s[:], scalar1=float(c[0]), scalar2=float(c[1]),
                    op0=Mult, op1=Add,
                )
                v = bpool.tile([P, width], iodt, tag="bv")
                nc.vector.tensor_mul(v[:], u[:], s[:])
                if BDEG == 3:
                    w_ = bpool.tile([P, width], iodt, tag="bw")
                    nc.vector.tensor_scalar_add(w_[:], v[:], float(c[2]))
                    p_ = bpool.tile([P, width], iodt, tag="bp")
                    nc.vector.tensor_mul(p_[:], w_[:], s[:])
                    q = bpool.tile([P, width], iodt, tag="bq")
                    nc.vector.tensor_scalar_add(q[:], p_[:], float(c[3]))
                else:
                    q = bpool.tile([P, width], iodt, tag="bq")
                    nc.vector.tensor_scalar_add(q[:], v[:], float(c[2]))
                ot = opool.tile([P, width], iodt, tag="ot")
                nc.vector.tensor_mul(ot[:], q[:], m[:])
                nc.gpsimd.dma_start(out=y_flat[:, t0 : t0 + width], in_=ot[:])

            # --- leading edge block (route A) ---
            th = apool.tile([P, EDGE], iodt, tag="th_e")
            sg = apool.tile([P, EDGE], iodt, tag="sg_e")
            run_blocks(
                xt0, 0,
                [(EDGE, [(Tanh, 1.0, th), (Sigmoid, 1.0, sg)])],
                None, 0,
            )
            route_a_finish(th, sg, EDGE, 0)

            # --- 5 middle pairs of BLK blocks ---
            for pair in range(N_MID // 2):
                t0 = EDGE + pair * 2 * BLK
                xt = xpool.tile([P, 2 * BLK + DIL], iodt, tag="xt")
                nc.sync.dma_start(out=xt[:], in_=x_flat[:, t0 - DIL : t0 + 2 * BLK])
                if pair in B_PAIRS:
                    t = bpool.tile([P, 2 * BLK], iodt, tag="bt")
                    for half in range(2):
                        ps = psumpool.tile([P, BLK], F32, tag="ps")
                        for k in (1, 0):
                            for c in range(0, BLK, MM_FD):
                                nc.tensor.matmul(
                                    out=ps[:, c : c + MM_FD],
                                    lhsT=wblk[k][:],
                                    rhs=xt[
                                        :,
                                        half * BLK + c + k * DIL : half * BLK + c + k * DIL + MM_FD,
                                    ],
                                    start=(k == 1),
                                    stop=(k == 0),
                                )
                        for _ in range(2):
                            nc.tensor.matmul(
                                out=psd[:, 0:128], lhsT=wdum[:], rhs=wdum[:],
                                start=True, stop=True,
                            )
                        nc.scalar.activation(
                            out=t[:, half * BLK : (half + 1) * BLK], in_=ps[:],
                            func=Tanh, scale=0.5,
                        )
                    route_b_finish(t, 2 * BLK, t0)
                else:
                    th = apool.tile([P, 2 * BLK], iodt, tag="th")
                    sg = apool.tile([P, 2 * BLK], iodt, tag="sg")
                    for half in range(2):
                        ps = psumpool.tile([P, BLK], F32, tag="ps")
                        for k in (1, 0):
                            for c in range(0, BLK, MM_FD):
                                nc.tensor.matmul(
                                    out=ps[:, c : c + MM_FD],
                                    lhsT=wblk[k][:],
                                    rhs=xt[
                                        :,
                                        half * BLK + c + k * DIL : half * BLK + c + k * DIL + MM_FD,
                                    ],
                                    start=(k == 1),
                                    stop=(k == 0),
                                )
                        for _ in range(2):
                            nc.tensor.matmul(
                                out=psd[:, 0:128], lhsT=wdum[:], rhs=wdum[:],
                                start=True, stop=True,
                            )
                        nc.scalar.activation(
                            out=th[:, half * BLK : (half + 1) * BLK], in_=ps[:],
                            func=Tanh,
                        )
                        nc.scalar.activation(
                            out=sg[:, half * BLK : (half + 1) * BLK], in_=ps[:],
                            func=Sigmoid,
                        )
                    route_a_finish(th, sg, 2 * BLK, t0)

            # --- trailing edge block (route A) ---
            t0 = T - EDGE
            xt = xpool.tile([P, EDGE + DIL], iodt, tag="xt_e")
            nc.sync.dma_start(out=xt[:], in_=x_flat[:, t0 - DIL : t0 + EDGE])
            th = apool.tile([P, EDGE], iodt, tag="th_e")
            sg = apool.tile([P, EDGE], iodt, tag="sg_e")
            run_blocks(
                xt, t0,
                [(EDGE, [(Tanh, 1.0, th), (Sigmoid, 1.0, sg)])],
                None, 0,
            )
            route_a_finish(th, sg, EDGE, t0)
    nc.finalize()
    return nc


def _host_weights(w: np.ndarray, np_dtype) -> np.ndarray:
    wt = np.zeros((KERNEL, P, P), dtype=np_dtype)
    for k in range(KERNEL):
        wTk = np.ascontiguousarray(w[:, :, k].T).astype(np_dtype)  # [ci, co]
        for b in range(B_PER):
            wt[k, b * C : (b + 1) * C, b * C : (b + 1) * C] = wTk
    return wt


def _ensure_ntff_hook():
    """Recreate the antenv.axon_hooks NTFF profiling shim if the image lacks it."""
    import types

    try:
        import antenv.axon_hooks  # noqa: F401

        return
    except ImportError:
        pass
    import antenv

    mod = types.ModuleType("antenv.axon_hooks")
    _h = {"hook": None}
    mod.set_axon_ntff_profile_hook = lambda h: _h.__setitem__("hook", h)
    mod.get_axon_ntff_profile_hook = lambda: _h["hook"]
    sys.modules["antenv.axon_hooks"] = mod
    antenv.axon_hooks = mod
    try:
        from trn_agent_boot.trn_boot import _ntff_profile_via_ctypes

        hook = _ntff_profile_via_ctypes("/opt/axon/libaxon_pjrt.so")
        if hook is not None:
            mod.set_axon_ntff_profile_hook(hook)
    except Exception as e:  # degrade to no-trace rather than crash
        print(f"ntff hook setup failed: {e}", file=sys.stderr)


def _run_spmd(x: np.ndarray, w: np.ndarray, trace: bool = False):
    from concourse import bass_utils
    from concourse.bass_utils import run_bass_kernel_spmd

    if trace:
        _ensure_ntff_hook()
        bass_utils.upload_artifacts = lambda tmpdir: tmpdir

    nc = _build_program()
    if IO_DTYPE == "bfloat16":
        import ml_dtypes

        np_dtype = ml_dtypes.bfloat16
    else:
        np_dtype = np.float16
    xio = np.ascontiguousarray(x.astype(np_dtype))
    wt = _host_weights(w, np_dtype)
    in_maps = [
        {"x": np.ascontiguousarray(xio[i * B_PER : (i + 1) * B_PER]), "wt": wt}
        for i in range(N_CORES)
    ]
    kwargs = {}
    if trace:
        import tempfile

        os.makedirs("/tmp/kernel_trace", exist_ok=True)
        kwargs["tmpdir"] = tempfile.mkdtemp(dir="/tmp/kernel_trace")
    res = run_bass_kernel_spmd(nc, in_maps, list(range(N_CORES)), trace=trace, **kwargs)
    y = np.concatenate(
        [res.results[i]["y"].astype(np.float32) for i in range(N_CORES)], axis=0
    )
    return y, res


def kernel(x: np.ndarray, w: np.ndarray) -> np.ndarray:
    x = np.ascontiguousarray(np.asarray(x, dtype=np.float32))
    w = np.ascontiguousarray(np.asarray(w, dtype=np.float32))
    trace = os.environ.get("KERNEL_TRACE", "0") == "1"
    y, res = _run_spmd(x, w, trace=trace)
    if trace:
        global LAST_RESULTS
        LAST_RESULTS = res
    return y


LAST_RESULTS = None
